# revision 1
# baseline (speedup 1.0000x reference)
"""Trainium2 Bass kernel for nn_Encoding (VQ codebook encoding).

Computation (per batch b):
    xd = x[b] viewed as (C, N) in DRAM, N = H*W
    dist = scale_k * (||x_n||^2 + ||c_k||^2 - 2 x_n . c_k)
    A = softmax_k(dist)
    encoded[b] = A^T @ xd^T - (sum_n A)[:, None] * codewords

Strategy: data-parallel over batch across 8 NeuronCores (8 images per core).
Host prep (cheap, O(B*C*N) numpy): bf16 copies of x in (C,N) and (N,C)
layouts (same total HBM bytes as one fp32 copy), exact fp32 x_sq, and the
softmax constants folded per-k (shift by s_max*x_sq keeps exp in range and
is mathematically exact for softmax).

Per image on-device:
  m1:    psum_xc(32,784)  = W1^T @ xb16         W1 = -2*s_k*cw (bf16), PE
  e:     e = x_sq_rep*sp_k + psum_xc            DVE (scalar_tensor_tensor)
  E:     E = exp(e + bias_k)                    ACT, bias_k = s_k*||c_k||^2
  den:   psum_den(32,784) = ones32^T @ E        PE fp32r (replicates sum_k)
  r:     r = 1/psum_den                         DVE
  A16:   A16 = E*r (bf16), wsum = sum_n         DVE tensor_tensor_reduce
  At:    psum_at = A16^T (7 PE transposes)      PE -> ACT copy to SBUF
  m2:    psum_wx(32,512) = At^T @ xT16          PE bf16, accumulate 7 chunks
  out:   enc = (-cw)*wsum + psum_wx             DVE scalar_tensor_tensor
"""

import os
from contextlib import ExitStack

import numpy as np
import ml_dtypes

import concourse.bass as bass
import concourse.bacc as bacc
import concourse.tile as tile
import concourse.mybir as mybir
import concourse.bass_utils as bass_utils

BF16 = ml_dtypes.bfloat16
F32 = mybir.dt.float32
F32R = mybir.dt.float32r
BF = mybir.dt.bfloat16

B, C, H, W = 64, 512, 28, 28
N = H * W            # 784
K = 32
NCORES = 8
BPC = B // NCORES    # 8 images per core
CCH = C // 128       # 4 c-chunks
NT = 7               # n-chunks for m2 / transposes
NC_ = N // NT        # 112
PIECES = ((0, 448), (448, 336))  # n-pieces: 4 chunks + 3 chunks

LAST_EXEC_NS = None
LAST_RESULTS = None


def _pin_act_table():
    """Make every activation func we use resolve to the single table set
    that contains all of them (Exp, Ln, Copy, Identity), so the ACT engine
    never reloads its function table mid-kernel (~1.3us per reload).
    We only mutate the cached selector sets — table ids/indices and the
    runtime table contents are untouched."""
    from concourse.hw_specs import get_activation_tables

    AF = mybir.ActivationFunctionType
    need = {AF.Exp, AF.Ln, AF.Copy, AF.Identity}
    tabs = get_activation_tables("gen3")
    if "natural_log_exp_and_others" in tabs:
        for name, s in tabs.items():
            if name != "natural_log_exp_and_others":
                s -= need


def build_nc():
    _pin_act_table()
    nc = bacc.Bacc(
        "TRN2", target_bir_lowering=False, debug=False, enable_asserts=False
    )
    xb = nc.dram_tensor("xb", [BPC, CCH, 128, N], BF, kind="ExternalInput").ap()
    xt = nc.dram_tensor("xt", [BPC, NT, NC_, C], BF, kind="ExternalInput").ap()
    xsq = nc.dram_tensor("xsq", [BPC, 3, N], BF, kind="ExternalInput").ap()
    w1 = nc.dram_tensor("w1", [128, CCH * K], BF, kind="ExternalInput").ap()
    sp3 = nc.dram_tensor("sp3", [3, K], BF, kind="ExternalInput").ap()
    spb = nc.dram_tensor("spb", [K, 2], F32, kind="ExternalInput").ap()
    negcw = nc.dram_tensor("negcw", [K, C], F32, kind="ExternalInput").ap()
    onec = nc.dram_tensor("onec", [NC_, 1], BF, kind="ExternalInput").ap()
    ident = nc.dram_tensor("ident", [K, K], BF, kind="ExternalInput").ap()
    enc = nc.dram_tensor("enc", [BPC, K, C], F32, kind="ExternalOutput").ap()

    with tile.TileContext(nc) as tc, ExitStack() as ctx:
        build_kernel(ctx, tc, xb, xt, xsq, w1, sp3, spb, negcw, onec, ident, enc)
    nc.compile()
    return nc


def build_kernel(ctx, tc, xb, xt, xsq, w1, sp3, spb, negcw, onec, ident, enc):
    nc = tc.nc
    consts = ctx.enter_context(tc.tile_pool(name="consts", bufs=1))
    xb_pool = ctx.enter_context(tc.tile_pool(name="xb", bufs=3))
    xt_pool = ctx.enter_context(tc.tile_pool(name="xt", bufs=3))
    sm_pool = ctx.enter_context(tc.tile_pool(name="sm", bufs=3))
    out_pool = ctx.enter_context(tc.tile_pool(name="out", bufs=2))
    ps_xc = ctx.enter_context(tc.tile_pool(name="ps_xc", bufs=4, space="PSUM"))
    ps_at = ctx.enter_context(tc.tile_pool(name="ps_at", bufs=2, space="PSUM"))
    ps_wx = ctx.enter_context(tc.tile_pool(name="ps_wx", bufs=2, space="PSUM"))

    # constants, loaded once
    w1_t = consts.tile([128, CCH * K], BF)
    nc.sync.dma_start(w1_t[:], w1)
    sp3_t = consts.tile([3, K], BF)
    nc.sync.dma_start(sp3_t[:], sp3)
    spb_t = consts.tile([K, 2], F32)
    nc.sync.dma_start(spb_t[:], spb)
    negcw_t = consts.tile([K, C], F32)
    nc.sync.dma_start(negcw_t[:], negcw)
    onec_t = consts.tile([NC_, 1], BF)
    nc.sync.dma_start(onec_t[:], onec)
    id_t = consts.tile([K, K], BF)
    nc.sync.dma_start(id_t[:], ident)

    for b in range(BPC):
        # ---- loads ----
        xb_t = xb_pool.tile([128, CCH * N], BF, tag="xb")
        nc.sync.dma_start(xb_t[:], xb[b].transpose((1, 0, 2)))
        xt_t = xt_pool.tile([NC_, NT * C], BF, tag="xt")
        nc.sync.dma_start(xt_t[:], xt[b].transpose((1, 0, 2)))
        xsq_t = sm_pool.tile([3, N], BF, tag="xsq")
        nc.sync.dma_start(xsq_t[:], xsq[b])

        # ---- m1 per n-piece: xcW = W1^T @ x + sp3^T @ xsq3, then exp ----
        # The sp_k*xsq_n logit term rides the same accumulation as a 3-row
        # bf16 matmul (hi/lo split of sp and xsq for fp32-grade accuracy).
        E_ts = []
        et_p = ps_at.tile([NC_, NT * K + 2], BF, tag="at")
        for off, nn_ in PIECES:
            xc_p = ps_xc.tile([K, 448], F32, tag="xc")
            for j in range(CCH):
                nc.tensor.matmul(
                    xc_p[:, :nn_],
                    w1_t[:, j * K : (j + 1) * K],
                    xb_t[:, j * N + off : j * N + off + nn_],
                    start=(j == 0),
                    stop=False,
                )
            nc.tensor.matmul(
                xc_p[:, :nn_],
                sp3_t[:],
                xsq_t[:, off : off + nn_],
                start=False,
                stop=True,
            )
            E_t = sm_pool.tile([K, 448], BF, tag="E")
            nc.scalar.activation(
                E_t[:, :nn_], xc_p[:, :nn_], mybir.ActivationFunctionType.Exp,
                bias=spb_t[:, 1:2], scale=1.0,
            )
            E_ts.append(E_t)
            # transposes for this piece (chunks of 112)
            for j in range(off // NC_, (off + nn_) // NC_):
                nc.tensor.transpose(
                    et_p[:, j * K : (j + 1) * K],
                    E_t[:, j * NC_ - off : (j + 1) * NC_ - off],
                    id_t[:],
                )

        # ---- per-n denom + normalize in (n, k) layout (all from PSUM) ----
        d_t = sm_pool.tile([NC_, NT], F32, tag="d")
        nc.vector.reduce_sum(
            d_t[:], et_p[:, : NT * K].rearrange("p (j k) -> p j k", k=K),
            axis=mybir.AxisListType.X,
        )
        r_t = sm_pool.tile([NC_, NT], F32, tag="r")
        nc.vector.reciprocal(r_t[:], d_t[:])
        at_t = sm_pool.tile([NC_, NT * K], BF, tag="ats")
        nc.vector.tensor_mul(
            at_t[:].rearrange("p (j k) -> p j k", k=K),
            et_p[:, : NT * K].rearrange("p (j k) -> p j k", k=K),
            r_t[:].unsqueeze(-1).broadcast_to((NC_, NT, K)),
        )

        # ---- m2: wx = A^T^T @ xT; wsum rides in the et_p bank (bitcast) ----
        wx_p = ps_wx.tile([K, C], F32, tag="wx")
        ws_p = et_p[0:K, NT * K : NT * K + 2].bitcast(F32)
        for j in range(NT):
            nc.tensor.matmul(
                wx_p[:],
                at_t[:, j * K : (j + 1) * K],
                xt_t[:, j * C : (j + 1) * C],
                start=(j == 0),
                stop=(j == NT - 1),
            )
            nc.tensor.matmul(
                ws_p,
                at_t[:, j * K : (j + 1) * K],
                onec_t[:],
                start=(j == 0),
                stop=(j == NT - 1),
            )

        # ---- enc = (-cw)*wsum + wx ----
        o_t = out_pool.tile([K, C], F32, tag="o")
        nc.vector.scalar_tensor_tensor(
            o_t[:], negcw_t[:], ws_p, wx_p[:],
            op0=mybir.AluOpType.mult, op1=mybir.AluOpType.add,
        )
        nc.sync.dma_start(enc[b], o_t[:])


def host_prep(x, codewords, scale):
    """Build per-core input maps. x:(64,512,28,28) cw:(32,512) s:(32,)"""
    x = np.asarray(x, np.float32).reshape(B, C, N)
    cw = np.asarray(codewords, np.float32)
    s = np.asarray(scale, np.float32)

    s_max = float(s.max())
    sp = (s - s_max).astype(np.float32)
    c_sq = (cw * cw).sum(-1)
    bias = (s * c_sq).astype(np.float32)
    spb = np.stack([sp, bias], axis=1).astype(np.float32)  # (K, 2)
    sph = sp.astype(BF16)
    spl = (sp - sph.astype(np.float32)).astype(BF16)
    sp3 = np.stack([sph, sph, spl], axis=0)  # (3, K) bf16

    w1_full = (-2.0 * s[None, :] * cw.T).astype(np.float32)  # (C, K)
    w1 = np.ascontiguousarray(
        w1_full.reshape(CCH, 128, K).transpose(1, 0, 2).reshape(128, CCH * K)
    ).astype(BF16)
    negcw = np.ascontiguousarray(-cw).astype(np.float32)
    onec = np.ones((NC_, 1), BF16)
    ident = np.eye(K).astype(BF16)

    xb_all = x.reshape(B, CCH, 128, N).astype(BF16)  # (B,4,128,784)
    xt_all = np.ascontiguousarray(x.transpose(0, 2, 1)).reshape(
        B, NT, NC_, C
    ).astype(BF16)
    xsq_f32 = (x * x).sum(1).astype(np.float32)  # (B, 784)
    xh = xsq_f32.astype(BF16)
    xl = (xsq_f32 - xh.astype(np.float32)).astype(BF16)
    xsq_all = np.stack([xh, xl, xh], axis=1)  # (B, 3, 784) bf16 rows [xh,xl,xh]

    in_maps = []
    for i in range(NCORES):
        sl = slice(i * BPC, (i + 1) * BPC)
        in_maps.append(
            {
                "xb": np.ascontiguousarray(xb_all[sl]),
                "xt": np.ascontiguousarray(xt_all[sl]),
                "xsq": np.ascontiguousarray(xsq_all[sl]),
                "sp3": sp3,
                "w1": w1,
                "spb": spb,
                "negcw": negcw,
                "onec": onec,
                "ident": ident,
            }
        )
    return in_maps


_CACHED_NC = None


def _install_profile_shim():
    """Provide antenv.axon_hooks (absent in this container) so
    run_bass_kernel_spmd(trace=True) can NTFF-profile via the axon .so.
    Mirrors trn_agent_boot._ntff_profile_via_ctypes."""
    import sys
    import types
    import ctypes
    import contextlib

    if "antenv.axon_hooks" in sys.modules:
        return
    so_path = "/opt/axon/libaxon_pjrt.so"
    try:
        lib = ctypes.CDLL(so_path)
        if not hasattr(lib, "axon_start_nrt_profile"):
            return
    except OSError:
        return
    lib.axon_start_nrt_profile.argtypes = [
        ctypes.POINTER(ctypes.c_int64),
        ctypes.c_size_t,
    ]
    lib.axon_start_nrt_profile.restype = ctypes.c_int64
    lib.axon_stop_nrt_profile.argtypes = [ctypes.c_char_p]
    lib.axon_stop_nrt_profile.restype = ctypes.c_int64

    @contextlib.contextmanager
    def _hook(output_dir, device_ids):
        import jax

        jax.devices()
        if device_ids:
            ids = (ctypes.c_int64 * len(device_ids))(*device_ids)
            rc = lib.axon_start_nrt_profile(ids, len(device_ids))
        else:
            rc = lib.axon_start_nrt_profile(None, 0)
        if rc != 0:
            raise RuntimeError(f"axon_start_nrt_profile rc={rc}")
        try:
            yield
        finally:
            n = lib.axon_stop_nrt_profile(str(output_dir).encode())
            print(f"profile: {n} file(s) written to {output_dir}")

    mod = types.ModuleType("antenv.axon_hooks")
    mod.get_axon_ntff_profile_hook = lambda: _hook
    mod.set_axon_ntff_profile_hook = lambda h: None
    sys.modules["antenv.axon_hooks"] = mod
    import antenv

    antenv.axon_hooks = mod
    # skip bucket upload of artifacts (no bucket access here)
    bass_utils.upload_artifacts = lambda tmpdir: "local://" + tmpdir


def kernel(x, codewords, scale):
    global _CACHED_NC, LAST_EXEC_NS, LAST_RESULTS
    if _CACHED_NC is None:
        _CACHED_NC = build_nc()
    nc = _CACHED_NC
    in_maps = host_prep(x, codewords, scale)
    trace = bool(int(os.environ.get("KERNEL_TRACE", "0")))
    if trace:
        _install_profile_shim()
    res = bass_utils.run_bass_kernel_spmd(
        nc, in_maps, list(range(NCORES)), trace=trace
    )
    LAST_EXEC_NS = res.exec_time_ns
    LAST_RESULTS = res
    out = np.concatenate([res.results[i]["enc"] for i in range(NCORES)], axis=0)
    return out.astype(np.float32)



# revision 10
# speedup vs baseline: 1.1009x; 1.1009x over previous
"""Trainium2 Bass kernel for nn_Encoding (VQ codebook encoding).

Computation (per batch b, N = H*W = 784 pixels, K = 32 codes, C = 512):
    logit[n,k] = sp_k*xsq_n - 2 s_k (x_n . c_k) + s_k*||c_k||^2   (sp = s - s_max)
    A = softmax_k(logit)
    enc[k,c] = sum_n A[n,k]*x[n,c] - (sum_n A[n,k]) * cw[k,c]

Strategy: data-parallel over batch across 8 NeuronCores (8 images per core).

Per image on device (all matmuls keep x as the LDWEIGHTS stationary stream):
  m1:   lg_psum[n(112),k(32)] per n-chunk j: 4 accumulating fp8 matmuls with
        lhsT = xb chunk [128c, 112n] (fp8), rhs = 64*W1[128,32] (fp8, scaled
        out of the e4m3 subnormal range); a 5th 4-row bf16 matmul rides the
        softmax constants exactly:
          rows [xh, xl, xh, 1] x 64*[sph, sph, spl, bias_k]
          = 64*(sp_k*xsq_n (fp32-grade hi/lo) + s_k*||c_k||^2)
  exp:  E = exp(lg/64)                   ACT scale=1/64, (n,k) layout
  den:  den[n,j] = sum_k E; r = 1/den    DVE
  at:   at = E*r (bf16)                  DVE
  m2:   wx_psum[32,512] += sum_j at[j]^T @ xt[j]   bf16, at stationary
        ws_psum[32,1] rides the same stationaries against a ones vector
  out:  enc[32,512](bf16) = negcw*ws + wx   on GpSimd (Pool)

Images are software-pipelined with skew 2 (m2 for image b issues after m1 of
image b+2) so the PE never waits on the softmax round-trip.
"""

import os
from contextlib import ExitStack

import numpy as np
import ml_dtypes

import concourse.bass as bass
import concourse.bacc as bacc
import concourse.tile as tile
import concourse.mybir as mybir
import concourse.bass_utils as bass_utils

BF16 = ml_dtypes.bfloat16
FP8 = ml_dtypes.float8_e4m3fn
F32 = mybir.dt.float32
BF = mybir.dt.bfloat16
F8 = mybir.dt.float8e4

B, C, H, W = 64, 512, 28, 28
N = H * W            # 784
K = 32
NCORES = 8
BPC = B // NCORES    # 8 images per core
CCH = C // 128       # 4 c-chunks
NT = 7               # n-chunks
NC_ = N // NT        # 112
SKEW = 2             # m2 trails m1 by this many images
W1SC = 64.0          # fp8 scale for W1 (values would be e4m3-subnormal)

LAST_EXEC_NS = None
LAST_RESULTS = None


def _pin_act_table():
    """Make every activation func we use resolve to the single table set
    that contains all of them, so the ACT engine never reloads its function
    table mid-kernel (~1.3us per reload)."""
    from concourse.hw_specs import get_activation_tables

    AF = mybir.ActivationFunctionType
    need = {AF.Exp, AF.Ln, AF.Copy, AF.Identity}
    tabs = get_activation_tables("gen3")
    if "natural_log_exp_and_others" in tabs:
        for name, s in tabs.items():
            if name != "natural_log_exp_and_others":
                s -= need


def build_nc():
    _pin_act_table()
    nc = bacc.Bacc(
        "TRN2", target_bir_lowering=False, debug=False, enable_asserts=False
    )
    xb = nc.dram_tensor("xb", [BPC, 128, CCH * N], F8, kind="ExternalInput").ap()
    xt = nc.dram_tensor("xt", [BPC, NC_, NT * C], BF, kind="ExternalInput").ap()
    xq = nc.dram_tensor("xq", [BPC, 4, N], BF, kind="ExternalInput").ap()
    w1 = nc.dram_tensor("w1", [128, CCH * K], F8, kind="ExternalInput").ap()
    sp4 = nc.dram_tensor("sp4", [4, K], BF, kind="ExternalInput").ap()
    negcw = nc.dram_tensor("negcw", [K, C], F32, kind="ExternalInput").ap()
    onec = nc.dram_tensor("onec", [NC_, 1], BF, kind="ExternalInput").ap()
    enc = nc.dram_tensor("enc", [BPC, K, C], BF, kind="ExternalOutput").ap()

    with tile.TileContext(nc) as tc, ExitStack() as ctx:
        build_kernel(ctx, tc, xb, xt, xq, w1, sp4, negcw, onec, enc)
    nc.compile()
    return nc


def build_kernel(ctx, tc, xb, xt, xq, w1, sp4, negcw, onec, enc):
    nc = tc.nc
    consts = ctx.enter_context(tc.tile_pool(name="consts", bufs=1))
    xb_pool = ctx.enter_context(tc.tile_pool(name="xb", bufs=4))
    xt_pool = ctx.enter_context(tc.tile_pool(name="xt", bufs=4))
    xq_pool = ctx.enter_context(tc.tile_pool(name="xq", bufs=4))
    sm_pool = ctx.enter_context(tc.tile_pool(name="sm", bufs=2))
    at_pool = ctx.enter_context(tc.tile_pool(name="at", bufs=3))
    out_pool = ctx.enter_context(tc.tile_pool(name="out", bufs=2))
    ps_lg = ctx.enter_context(tc.tile_pool(name="ps_lg", bufs=3, space="PSUM"))
    ps_wx = ctx.enter_context(tc.tile_pool(name="ps_wx", bufs=2, space="PSUM"))
    ps_ws = ctx.enter_context(tc.tile_pool(name="ps_ws", bufs=2, space="PSUM"))

    # constants, loaded once
    w1_t = consts.tile([128, CCH * K], F8)
    nc.sync.dma_start(w1_t[:], w1)
    sp4_t = consts.tile([4, K], BF)
    nc.sync.dma_start(sp4_t[:], sp4)
    negcw_t = consts.tile([K, C], F32)
    nc.sync.dma_start(negcw_t[:], negcw)
    onec_t = consts.tile([NC_, 1], BF)
    nc.sync.dma_start(onec_t[:], onec)

    inflight = []
    for it in range(BPC + SKEW):
        if it < BPC:
            b = it
            # ---- loads ----
            xb_t = xb_pool.tile([128, CCH * N], F8, tag="xb")
            nc.sync.dma_start(xb_t[:], xb[b])
            xt_t = xt_pool.tile([NC_, NT * C], BF, tag="xt")
            nc.sync.dma_start(xt_t[:], xt[b])
            xq_t = xq_pool.tile([4, N], BF, tag="xq")
            nc.sync.dma_start(xq_t[:], xq[b])

            # ---- m1: logits in (n, k) layout; x is the stationary ----
            lg_p = ps_lg.tile([NC_, NT * K], F32, tag="lg")
            for j in range(NT):
                o = lg_p[:, j * K : (j + 1) * K]
                for jc in range(CCH):
                    nc.tensor.matmul(
                        o,
                        xb_t[:, jc * N + j * NC_ : jc * N + (j + 1) * NC_],
                        w1_t[:, jc * K : (jc + 1) * K],
                        start=(jc == 0),
                        stop=False,
                    )
                nc.tensor.matmul(
                    o,
                    xq_t[:, j * NC_ : (j + 1) * NC_],
                    sp4_t[:],
                    start=False,
                    stop=True,
                )

            # ---- softmax in (n, k): exp, denom over free dim, normalize ----
            E_t = sm_pool.tile([NC_, NT * K], BF, tag="E")
            nc.scalar.activation(
                E_t[:], lg_p[:], mybir.ActivationFunctionType.Exp,
                scale=1.0 / W1SC,
            )
            d_t = sm_pool.tile([NC_, NT], F32, tag="d")
            nc.vector.reduce_sum(
                d_t[:], E_t[:].rearrange("p (j k) -> p j k", k=K),
                axis=mybir.AxisListType.X,
            )
            r_t = sm_pool.tile([NC_, NT], F32, tag="r")
            nc.vector.reciprocal(r_t[:], d_t[:])
            at_t = at_pool.tile([NC_, NT * K], BF, tag="at")
            nc.vector.tensor_mul(
                at_t[:].rearrange("p (j k) -> p j k", k=K),
                E_t[:].rearrange("p (j k) -> p j k", k=K),
                r_t[:].unsqueeze(-1).broadcast_to((NC_, NT, K)),
            )
            inflight.append((b, xt_t, at_t))

        if it >= SKEW:
            b2, xt2, at2 = inflight.pop(0)
            xt2_v = xt2[:].rearrange("p (j c) -> p j c", c=C)
            wx_p = ps_wx.tile([K, C], F32, tag="wx")
            ws_p = ps_ws.tile([K, 1], F32, tag="ws")
            for j in range(NT):
                lhs = at2[:, j * K : (j + 1) * K]
                nc.tensor.matmul(
                    wx_p[:],
                    lhs,
                    xt2_v[:, j],
                    start=(j == 0),
                    stop=(j == NT - 1),
                )
                nc.tensor.matmul(
                    ws_p[:],
                    lhs,
                    onec_t[:],
                    start=(j == 0),
                    stop=(j == NT - 1),
                )
            # ---- enc = (-cw)*wsum + wx; DMA out ----
            o_t = out_pool.tile([K, C], BF, tag="o")
            nc.vector.scalar_tensor_tensor(
                o_t[:], negcw_t[:], ws_p[:], wx_p[:],
                op0=mybir.AluOpType.mult, op1=mybir.AluOpType.add,
            )
            nc.sync.dma_start(enc[b2], o_t[:])


def host_prep(x, codewords, scale):
    """Build per-core input maps. x:(64,512,28,28) cw:(32,512) s:(32,)"""
    x = np.asarray(x, np.float32).reshape(B, C, N)
    cw = np.asarray(codewords, np.float32)
    s = np.asarray(scale, np.float32)

    s_max = float(s.max())
    sp = (s - s_max).astype(np.float32) * W1SC
    c_sq = (cw * cw).sum(-1)
    bias = (s * c_sq).astype(np.float32) * W1SC
    sph = sp.astype(BF16)
    spl = (sp - sph.astype(np.float32)).astype(BF16)
    sp4 = np.stack([sph, sph, spl, bias.astype(BF16)], axis=0)  # (4, K) bf16

    w1_full = (-2.0 * W1SC * s[None, :] * cw.T).astype(np.float32)  # (C, K)
    w1 = np.ascontiguousarray(
        w1_full.reshape(CCH, 128, K).transpose(1, 0, 2).reshape(128, CCH * K)
    ).astype(FP8)
    negcw = np.ascontiguousarray(-cw).astype(np.float32)
    onec = np.ones((NC_, 1), BF16)

    # xb[b, p, jc*N + n] = x[b, jc*128 + p, n]  (3136B contiguous per part)
    xb_all = np.ascontiguousarray(
        x.reshape(B, CCH, 128, N).transpose(0, 2, 1, 3)
    ).reshape(B, 128, CCH * N).astype(FP8)
    # xt[b, p, j*C + c] = x[b, c, j*112 + p]  (7168B contiguous per part)
    xt_all = np.ascontiguousarray(
        x.transpose(0, 2, 1).reshape(B, NT, NC_, C).transpose(0, 2, 1, 3)
    ).reshape(B, NC_, NT * C).astype(BF16)
    xsq_f32 = (x * x).sum(1).astype(np.float32)  # (B, 784)
    xh = xsq_f32.astype(BF16)
    xl = (xsq_f32 - xh.astype(np.float32)).astype(BF16)
    ones_n = np.ones_like(xh)
    xq_all = np.stack([xh, xl, xh, ones_n], axis=1)  # (B, 4, 784) bf16

    in_maps = []
    for i in range(NCORES):
        sl = slice(i * BPC, (i + 1) * BPC)
        in_maps.append(
            {
                "xb": np.ascontiguousarray(xb_all[sl]),
                "xt": np.ascontiguousarray(xt_all[sl]),
                "xq": np.ascontiguousarray(xq_all[sl]),
                "w1": w1,
                "sp4": sp4,
                "negcw": negcw,
                "onec": onec,
            }
        )
    return in_maps


_CACHED_NC = None


def _install_profile_shim():
    """Provide antenv.axon_hooks (absent in this container) so
    run_bass_kernel_spmd(trace=True) can NTFF-profile via the axon .so."""
    import sys
    import types
    import ctypes
    import contextlib

    if "antenv.axon_hooks" in sys.modules:
        return
    so_path = "/opt/axon/libaxon_pjrt.so"
    try:
        lib = ctypes.CDLL(so_path)
        if not hasattr(lib, "axon_start_nrt_profile"):
            return
    except OSError:
        return
    lib.axon_start_nrt_profile.argtypes = [
        ctypes.POINTER(ctypes.c_int64),
        ctypes.c_size_t,
    ]
    lib.axon_start_nrt_profile.restype = ctypes.c_int64
    lib.axon_stop_nrt_profile.argtypes = [ctypes.c_char_p]
    lib.axon_stop_nrt_profile.restype = ctypes.c_int64

    @contextlib.contextmanager
    def _hook(output_dir, device_ids):
        import jax

        jax.devices()
        if device_ids:
            ids = (ctypes.c_int64 * len(device_ids))(*device_ids)
            rc = lib.axon_start_nrt_profile(ids, len(device_ids))
        else:
            rc = lib.axon_start_nrt_profile(None, 0)
        if rc != 0:
            raise RuntimeError(f"axon_start_nrt_profile rc={rc}")
        try:
            yield
        finally:
            n = lib.axon_stop_nrt_profile(str(output_dir).encode())
            print(f"profile: {n} file(s) written to {output_dir}")

    mod = types.ModuleType("antenv.axon_hooks")
    mod.get_axon_ntff_profile_hook = lambda: _hook
    mod.set_axon_ntff_profile_hook = lambda h: None
    sys.modules["antenv.axon_hooks"] = mod
    import antenv

    antenv.axon_hooks = mod
    bass_utils.upload_artifacts = lambda tmpdir: "local://" + tmpdir


def kernel(x, codewords, scale):
    global _CACHED_NC, LAST_EXEC_NS, LAST_RESULTS
    if _CACHED_NC is None:
        _CACHED_NC = build_nc()
    nc = _CACHED_NC
    in_maps = host_prep(x, codewords, scale)
    trace = bool(int(os.environ.get("KERNEL_TRACE", "0")))
    if trace:
        _install_profile_shim()
    res = bass_utils.run_bass_kernel_spmd(
        nc, in_maps, list(range(NCORES)), trace=trace
    )
    LAST_EXEC_NS = res.exec_time_ns
    LAST_RESULTS = res
    out = np.concatenate(
        [np.asarray(res.results[i]["enc"]) for i in range(NCORES)], axis=0
    )
    return out.astype(np.float32)


# revision 12
# speedup vs baseline: 1.1513x; 1.0458x over previous
"""Trainium2 Bass kernel for nn_Encoding (VQ codebook encoding).

Computation (per batch b, N = H*W = 784 pixels, K = 32 codes, C = 512):
    logit[n,k] = sp_k*xsq_n - 2 s_k (x_n . c_k) + s_k*||c_k||^2   (sp = s - s_max)
    A = softmax_k(logit)
    enc[k,c] = sum_n A[n,k]*x[n,c] - (sum_n A[n,k]) * cw[k,c]

Strategy: data-parallel over batch across 8 NeuronCores (8 images per core).

Per image on device (all matmuls keep x as the LDWEIGHTS stationary stream):
  m1:   lg_psum[n(112),k(32)] per n-chunk j: 4 accumulating fp8 matmuls with
        lhsT = xb chunk [128c, 112n] (fp8), rhs = 64*W1[128,32] (fp8, scaled
        out of the e4m3 subnormal range); a 5th 4-row bf16 matmul rides the
        softmax constants exactly:
          rows [xh, xl, xh, 1] x 64*[sph, sph, spl, bias_k]
          = 64*(sp_k*xsq_n (fp32-grade hi/lo) + s_k*||c_k||^2)
  exp:  E = exp(lg/64)                   ACT scale=1/64, (n,k) layout
  den:  den[n,j] = sum_k E; r = 1/den    DVE
  at:   at = E*r (bf16)                  DVE
  m2:   wx_psum[32,512] += sum_j at[j]^T @ xt[j]   bf16, at stationary
        ws_psum[32,1] rides the same stationaries against a ones vector
  out:  enc[32,512](bf16) = negcw*ws + wx   on GpSimd (Pool)

Images are software-pipelined with skew 2 (m2 for image b issues after m1 of
image b+2) so the PE never waits on the softmax round-trip.
"""

import os
from contextlib import ExitStack

import numpy as np
import ml_dtypes

import concourse.bass as bass
import concourse.bacc as bacc
import concourse.tile as tile
import concourse.mybir as mybir
import concourse.bass_utils as bass_utils

BF16 = ml_dtypes.bfloat16
FP8 = ml_dtypes.float8_e4m3fn
F32 = mybir.dt.float32
BF = mybir.dt.bfloat16
F8 = mybir.dt.float8e4

B, C, H, W = 64, 512, 28, 28
N = H * W            # 784
K = 32
NCORES = 8
BPC = B // NCORES    # 8 images per core
CCH = C // 128       # 4 c-chunks
NT = 7               # n-chunks
NC_ = N // NT        # 112
SKEW = 2             # m2 trails m1 by this many images
W1SC = 64.0          # fp8 scale for W1 (values would be e4m3-subnormal)

LAST_EXEC_NS = None
LAST_RESULTS = None


def _pin_act_table():
    """Make every activation func we use resolve to the single table set
    that contains all of them, so the ACT engine never reloads its function
    table mid-kernel (~1.3us per reload)."""
    from concourse.hw_specs import get_activation_tables

    AF = mybir.ActivationFunctionType
    need = {AF.Exp, AF.Ln, AF.Copy, AF.Identity}
    tabs = get_activation_tables("gen3")
    if "natural_log_exp_and_others" in tabs:
        for name, s in tabs.items():
            if name != "natural_log_exp_and_others":
                s -= need


def build_nc():
    _pin_act_table()
    nc = bacc.Bacc(
        "TRN2", target_bir_lowering=False, debug=False, enable_asserts=False
    )
    xb = nc.dram_tensor("xb", [BPC, 128, CCH * N], F8, kind="ExternalInput").ap()
    xt = nc.dram_tensor("xt", [BPC, NC_, NT * C], BF, kind="ExternalInput").ap()
    xq = nc.dram_tensor("xq", [BPC, 4, N], BF, kind="ExternalInput").ap()
    w1 = nc.dram_tensor("w1", [128, CCH * K], F8, kind="ExternalInput").ap()
    sp4 = nc.dram_tensor("sp4", [4, K], BF, kind="ExternalInput").ap()
    negcw = nc.dram_tensor("negcw", [K, C], F32, kind="ExternalInput").ap()
    onec = nc.dram_tensor("onec", [NC_, 1], BF, kind="ExternalInput").ap()
    enc = nc.dram_tensor("enc", [BPC, K, C], BF, kind="ExternalOutput").ap()

    with tile.TileContext(nc) as tc, ExitStack() as ctx:
        build_kernel(ctx, tc, xb, xt, xq, w1, sp4, negcw, onec, enc)
    nc.compile()
    return nc


def build_kernel(ctx, tc, xb, xt, xq, w1, sp4, negcw, onec, enc):
    nc = tc.nc
    consts = ctx.enter_context(tc.tile_pool(name="consts", bufs=1))
    xb_pool = ctx.enter_context(tc.tile_pool(name="xb", bufs=4))
    xt_pool = ctx.enter_context(tc.tile_pool(name="xt", bufs=4))
    xq_pool = ctx.enter_context(tc.tile_pool(name="xq", bufs=4))
    sm_pool = ctx.enter_context(tc.tile_pool(name="sm", bufs=4))
    at_pool = ctx.enter_context(tc.tile_pool(name="at", bufs=4))
    out_pool = ctx.enter_context(tc.tile_pool(name="out", bufs=3))
    ps_lga = ctx.enter_context(tc.tile_pool(name="ps_lga", bufs=3, space="PSUM"))
    ps_lgb = ctx.enter_context(tc.tile_pool(name="ps_lgb", bufs=3, space="PSUM"))
    ps_wx = ctx.enter_context(tc.tile_pool(name="ps_wx", bufs=2, space="PSUM"))

    # constants, loaded once
    w1_t = consts.tile([128, CCH * K], F8)
    nc.sync.dma_start(w1_t[:], w1)
    sp4_t = consts.tile([4, K], BF)
    nc.sync.dma_start(sp4_t[:], sp4)
    negcw_t = consts.tile([K, C], F32)
    nc.sync.dma_start(negcw_t[:], negcw)
    onec_t = consts.tile([NC_, 1], BF)
    nc.sync.dma_start(onec_t[:], onec)

    inflight = []
    for it in range(BPC + SKEW):
        if it < BPC:
            b = it
            # ---- loads ----
            xb_t = xb_pool.tile([128, CCH * N], F8, tag="xb")
            nc.sync.dma_start(xb_t[:], xb[b])
            xt_t = xt_pool.tile([NC_, NT * C], BF, tag="xt")
            nc.sync.dma_start(xt_t[:], xt[b])
            xq_t = xq_pool.tile([4, N], BF, tag="xq")
            nc.sync.dma_start(xq_t[:], xq[b])

            # ---- m1: logits in (n, k) layout; x is the stationary.
            # Chunks interleave across two PSUM banks (A: 0,2,4,6 / B: 1,3,5)
            # so the bf16 constant rides batch in pairs and the fp8 x-chunk
            # matmuls run in uninterrupted groups of 8 (fewer dtype switches).
            NA = 4  # chunks in bank A
            lga_p = ps_lga.tile([NC_, NA * K + 1], F32, tag="lga")
            lgb_p = ps_lgb.tile([NC_, (NT - NA) * K], F32, tag="lgb")

            def chunk_out(j):
                g = j // 2
                if j % 2 == 0:
                    return lga_p[:, g * K : (g + 1) * K]
                return lgb_p[:, g * K : (g + 1) * K]

            for jp in range(0, NT, 2):
                pair = [jp] + ([jp + 1] if jp + 1 < NT else [])
                for j in pair:  # bf16 rides open each chunk's psum group
                    nc.tensor.matmul(
                        chunk_out(j),
                        xq_t[:, j * NC_ : (j + 1) * NC_],
                        sp4_t[:],
                        start=True,
                        stop=False,
                    )
                for j in pair:  # fp8 x-chunk matmuls, grouped by dtype
                    o = chunk_out(j)
                    for jc in range(CCH):
                        nc.tensor.matmul(
                            o,
                            xb_t[:, jc * N + j * NC_ : jc * N + (j + 1) * NC_],
                            w1_t[:, jc * K : (jc + 1) * K],
                            start=False,
                            stop=(jc == CCH - 1),
                        )

            # ---- softmax in (n, k): exp, denom over free dim, normalize.
            # Chunk order in E/den/at is bank-grouped: [0,2,4,6, 1,3,5].
            Ea_t = sm_pool.tile([NC_, NA * K], BF, tag="Ea")
            Eb_t = sm_pool.tile([NC_, (NT - NA) * K], BF, tag="Eb")
            nc.scalar.activation(
                Eb_t[:], lgb_p[:], mybir.ActivationFunctionType.Exp,
                scale=1.0 / W1SC,
            )
            nc.scalar.activation(
                Ea_t[:], lga_p[:, : NA * K], mybir.ActivationFunctionType.Exp,
                scale=1.0 / W1SC,
            )
            d_t = sm_pool.tile([NC_, NT], F32, tag="d")
            nc.vector.reduce_sum(
                d_t[:, :NA], Ea_t[:].rearrange("p (j k) -> p j k", k=K),
                axis=mybir.AxisListType.X,
            )
            nc.vector.reduce_sum(
                d_t[:, NA:], Eb_t[:].rearrange("p (j k) -> p j k", k=K),
                axis=mybir.AxisListType.X,
            )
            r_t = sm_pool.tile([NC_, NT], F32, tag="r")
            nc.vector.reciprocal(r_t[:], d_t[:])
            at_t = at_pool.tile([NC_, NT * K], BF, tag="at")
            nc.vector.tensor_mul(
                at_t[:, : NA * K].rearrange("p (j k) -> p j k", k=K),
                Ea_t[:].rearrange("p (j k) -> p j k", k=K),
                r_t[:, :NA].unsqueeze(-1).broadcast_to((NC_, NA, K)),
            )
            nc.vector.tensor_mul(
                at_t[:, NA * K :].rearrange("p (j k) -> p j k", k=K),
                Eb_t[:].rearrange("p (j k) -> p j k", k=K),
                r_t[:, NA:].unsqueeze(-1).broadcast_to((NC_, NT - NA, K)),
            )
            inflight.append((b, xt_t, at_t, lga_p))

        if it >= SKEW:
            b2, xt2, at2, lga2 = inflight.pop(0)
            xt2_v = xt2[:].rearrange("p (j c) -> p j c", c=C)
            wx_p = ps_wx.tile([K, C], F32, tag="wx")
            ws_p = lga2[0:K, NA * K : NA * K + 1]  # rides in image b2's A bank
            for j in range(NT):
                g = j // 2 if j % 2 == 0 else NA + j // 2
                lhs = at2[:, g * K : (g + 1) * K]
                nc.tensor.matmul(
                    wx_p[:],
                    lhs,
                    xt2_v[:, j],
                    start=(j == 0),
                    stop=(j == NT - 1),
                )
                nc.tensor.matmul(
                    ws_p,
                    lhs,
                    onec_t[:],
                    start=(j == 0),
                    stop=(j == NT - 1),
                )
            # ---- enc = (-cw)*wsum + wx; DMA out ----
            o_t = out_pool.tile([K, C], BF, tag="o")
            nc.vector.scalar_tensor_tensor(
                o_t[:], negcw_t[:], ws_p, wx_p[:],
                op0=mybir.AluOpType.mult, op1=mybir.AluOpType.add,
            )
            nc.sync.dma_start(enc[b2], o_t[:])


def host_prep(x, codewords, scale):
    """Build per-core input maps. x:(64,512,28,28) cw:(32,512) s:(32,)"""
    x = np.asarray(x, np.float32).reshape(B, C, N)
    cw = np.asarray(codewords, np.float32)
    s = np.asarray(scale, np.float32)

    s_max = float(s.max())
    sp = (s - s_max).astype(np.float32) * W1SC
    c_sq = (cw * cw).sum(-1)
    bias = (s * c_sq).astype(np.float32) * W1SC
    sph = sp.astype(BF16)
    spl = (sp - sph.astype(np.float32)).astype(BF16)
    sp4 = np.stack([sph, sph, spl, bias.astype(BF16)], axis=0)  # (4, K) bf16

    w1_full = (-2.0 * W1SC * s[None, :] * cw.T).astype(np.float32)  # (C, K)
    w1 = np.ascontiguousarray(
        w1_full.reshape(CCH, 128, K).transpose(1, 0, 2).reshape(128, CCH * K)
    ).astype(FP8)
    negcw = np.ascontiguousarray(-cw).astype(np.float32)
    onec = np.ones((NC_, 1), BF16)

    # xb[b, p, jc*N + n] = x[b, jc*128 + p, n]  (3136B contiguous per part)
    xb_all = np.ascontiguousarray(
        x.reshape(B, CCH, 128, N).transpose(0, 2, 1, 3)
    ).reshape(B, 128, CCH * N).astype(FP8)
    # xt[b, p, j*C + c] = x[b, c, j*112 + p]  (7168B contiguous per part)
    xt_all = np.ascontiguousarray(
        x.transpose(0, 2, 1).reshape(B, NT, NC_, C).transpose(0, 2, 1, 3)
    ).reshape(B, NC_, NT * C).astype(BF16)
    xsq_f32 = (x * x).sum(1).astype(np.float32)  # (B, 784)
    xh = xsq_f32.astype(BF16)
    xl = (xsq_f32 - xh.astype(np.float32)).astype(BF16)
    ones_n = np.ones_like(xh)
    xq_all = np.stack([xh, xl, xh, ones_n], axis=1)  # (B, 4, 784) bf16

    in_maps = []
    for i in range(NCORES):
        sl = slice(i * BPC, (i + 1) * BPC)
        in_maps.append(
            {
                "xb": np.ascontiguousarray(xb_all[sl]),
                "xt": np.ascontiguousarray(xt_all[sl]),
                "xq": np.ascontiguousarray(xq_all[sl]),
                "w1": w1,
                "sp4": sp4,
                "negcw": negcw,
                "onec": onec,
            }
        )
    return in_maps


_CACHED_NC = None


def _install_profile_shim():
    """Provide antenv.axon_hooks (absent in this container) so
    run_bass_kernel_spmd(trace=True) can NTFF-profile via the axon .so."""
    import sys
    import types
    import ctypes
    import contextlib

    if "antenv.axon_hooks" in sys.modules:
        return
    so_path = "/opt/axon/libaxon_pjrt.so"
    try:
        lib = ctypes.CDLL(so_path)
        if not hasattr(lib, "axon_start_nrt_profile"):
            return
    except OSError:
        return
    lib.axon_start_nrt_profile.argtypes = [
        ctypes.POINTER(ctypes.c_int64),
        ctypes.c_size_t,
    ]
    lib.axon_start_nrt_profile.restype = ctypes.c_int64
    lib.axon_stop_nrt_profile.argtypes = [ctypes.c_char_p]
    lib.axon_stop_nrt_profile.restype = ctypes.c_int64

    @contextlib.contextmanager
    def _hook(output_dir, device_ids):
        import jax

        jax.devices()
        if device_ids:
            ids = (ctypes.c_int64 * len(device_ids))(*device_ids)
            rc = lib.axon_start_nrt_profile(ids, len(device_ids))
        else:
            rc = lib.axon_start_nrt_profile(None, 0)
        if rc != 0:
            raise RuntimeError(f"axon_start_nrt_profile rc={rc}")
        try:
            yield
        finally:
            n = lib.axon_stop_nrt_profile(str(output_dir).encode())
            print(f"profile: {n} file(s) written to {output_dir}")

    mod = types.ModuleType("antenv.axon_hooks")
    mod.get_axon_ntff_profile_hook = lambda: _hook
    mod.set_axon_ntff_profile_hook = lambda h: None
    sys.modules["antenv.axon_hooks"] = mod
    import antenv

    antenv.axon_hooks = mod
    bass_utils.upload_artifacts = lambda tmpdir: "local://" + tmpdir


def kernel(x, codewords, scale):
    global _CACHED_NC, LAST_EXEC_NS, LAST_RESULTS
    if _CACHED_NC is None:
        _CACHED_NC = build_nc()
    nc = _CACHED_NC
    in_maps = host_prep(x, codewords, scale)
    trace = bool(int(os.environ.get("KERNEL_TRACE", "0")))
    if trace:
        _install_profile_shim()
    res = bass_utils.run_bass_kernel_spmd(
        nc, in_maps, list(range(NCORES)), trace=trace
    )
    LAST_EXEC_NS = res.exec_time_ns
    LAST_RESULTS = res
    out = np.concatenate(
        [np.asarray(res.results[i]["enc"]) for i in range(NCORES)], axis=0
    )
    return out.astype(np.float32)


# revision 19
# speedup vs baseline: 1.1548x; 1.0030x over previous
"""Trainium2 Bass kernel for nn_Encoding (VQ codebook encoding).

Computation (per batch b, N = H*W = 784 pixels, K = 32 codes, C = 512):
    logit[n,k] = sp_k*xsq_n - 2 s_k (x_n . c_k) + s_k*||c_k||^2   (sp = s - s_max)
    A = softmax_k(logit)
    enc[k,c] = sum_n A[n,k]*x[n,c] - (sum_n A[n,k]) * cw[k,c]

Strategy: data-parallel over batch across 8 NeuronCores (8 images per core).

Per image on device (all matmuls keep x as the LDWEIGHTS stationary stream):
  m1:   lg_psum[n(112),k(32)] per n-chunk j: 4 accumulating fp8 matmuls with
        lhsT = xb chunk [128c, 112n] (fp8), rhs = 64*W1[128,32] (fp8, scaled
        out of the e4m3 subnormal range); a 5th 4-row bf16 matmul rides the
        softmax constants exactly:
          rows [xh, xl, xh, 1] x 64*[sph, sph, spl, bias_k]
          = 64*(sp_k*xsq_n (fp32-grade hi/lo) + s_k*||c_k||^2)
  exp:  E = exp(lg/64)                   ACT scale=1/64, (n,k) layout
  den:  den[n,j] = sum_k E; r = 1/den    DVE
  at:   at = E*r (bf16)                  DVE
  m2:   wx_psum[32,512] += sum_j at[j]^T @ xt[j]   bf16, at stationary
        ws_psum[32,1] rides the same stationaries against a ones vector
  out:  enc[32,512](bf16) = negcw*ws + wx   on GpSimd (Pool)

Images are software-pipelined with skew 2 (m2 for image b issues after m1 of
image b+2) so the PE never waits on the softmax round-trip.
"""

import os
from contextlib import ExitStack

import numpy as np
import ml_dtypes

import concourse.bass as bass
import concourse.bacc as bacc
import concourse.tile as tile
import concourse.mybir as mybir
import concourse.bass_utils as bass_utils

BF16 = ml_dtypes.bfloat16
FP8 = ml_dtypes.float8_e4m3fn
F32 = mybir.dt.float32
BF = mybir.dt.bfloat16
F8 = mybir.dt.float8e4

B, C, H, W = 64, 512, 28, 28
N = H * W            # 784
K = 32
NCORES = 8
BPC = B // NCORES    # 8 images per core
CCH = C // 128       # 4 c-chunks
NT = 7               # n-chunks
NC_ = N // NT        # 112
SKEW = 3             # m2 trails m1 by this many images
W1SC = 64.0          # fp8 scale for W1 (values would be e4m3-subnormal)

LAST_EXEC_NS = None
LAST_RESULTS = None


def _pin_act_table():
    """Make every activation func we use resolve to the single table set
    that contains all of them, so the ACT engine never reloads its function
    table mid-kernel (~1.3us per reload)."""
    from concourse.hw_specs import get_activation_tables

    AF = mybir.ActivationFunctionType
    need = {AF.Exp, AF.Ln, AF.Copy, AF.Identity}
    tabs = get_activation_tables("gen3")
    if "natural_log_exp_and_others" in tabs:
        for name, s in tabs.items():
            if name != "natural_log_exp_and_others":
                s -= need


def build_nc():
    _pin_act_table()
    nc = bacc.Bacc(
        "TRN2", target_bir_lowering=False, debug=False, enable_asserts=False
    )
    xb = nc.dram_tensor("xb", [BPC, 128, CCH * N], F8, kind="ExternalInput").ap()
    xt = nc.dram_tensor("xt", [BPC, NC_, NT * C], BF, kind="ExternalInput").ap()
    xq = nc.dram_tensor("xq", [BPC, 4, N], BF, kind="ExternalInput").ap()
    w1 = nc.dram_tensor("w1", [128, CCH * K], F8, kind="ExternalInput").ap()
    sp4 = nc.dram_tensor("sp4", [4, K], BF, kind="ExternalInput").ap()
    negcw = nc.dram_tensor("negcw", [K, C], F32, kind="ExternalInput").ap()
    onec = nc.dram_tensor("onec", [NC_, 1], BF, kind="ExternalInput").ap()
    enc = nc.dram_tensor("enc", [BPC, K, C], BF, kind="ExternalOutput").ap()

    with tile.TileContext(nc) as tc, ExitStack() as ctx:
        build_kernel(ctx, tc, xb, xt, xq, w1, sp4, negcw, onec, enc)
    nc.compile()
    return nc


def build_kernel(ctx, tc, xb, xt, xq, w1, sp4, negcw, onec, enc):
    nc = tc.nc
    consts = ctx.enter_context(tc.tile_pool(name="consts", bufs=1))
    xb_pool = ctx.enter_context(tc.tile_pool(name="xb", bufs=5))
    xt_pool = ctx.enter_context(tc.tile_pool(name="xt", bufs=5))
    xq_pool = ctx.enter_context(tc.tile_pool(name="xq", bufs=5))
    sm_pool = ctx.enter_context(tc.tile_pool(name="sm", bufs=4))
    at_pool = ctx.enter_context(tc.tile_pool(name="at", bufs=5))
    sb2_pool = ctx.enter_context(tc.tile_pool(name="sb2", bufs=3))
    out_pool = ctx.enter_context(tc.tile_pool(name="out", bufs=3))
    ps_lga = ctx.enter_context(tc.tile_pool(name="ps_lga", bufs=2, space="PSUM"))
    ps_lgb = ctx.enter_context(tc.tile_pool(name="ps_lgb", bufs=2, space="PSUM"))
    ps_wx = ctx.enter_context(tc.tile_pool(name="ps_wx", bufs=2, space="PSUM"))
    ps_ws = ctx.enter_context(tc.tile_pool(name="ps_ws", bufs=2, space="PSUM"))

    # constants, loaded once; negcw/onec go via the idle Pool DGE queue so
    # the SP queue reaches the first image's loads sooner
    w1_t = consts.tile([128, CCH * K], F8)
    nc.sync.dma_start(w1_t[:], w1)
    sp4_t = consts.tile([4, K], BF)
    nc.sync.dma_start(sp4_t[:], sp4)
    negcw_t = consts.tile([K, C], F32)
    nc.gpsimd.dma_start(negcw_t[:], negcw)
    onec_t = consts.tile([NC_, 1], BF)
    nc.gpsimd.dma_start(onec_t[:], onec)

    inflight = []
    for it in range(BPC + SKEW):
        if it < BPC:
            b = it
            # ---- loads (triggers spread across engine DGE queues so no
            # single sequencer serializes DMA issue) ----
            xb_t = xb_pool.tile([128, CCH * N], F8, tag="xb")
            nc.sync.dma_start(xb_t[:], xb[b])
            xt_t = xt_pool.tile([NC_, NT * C], BF, tag="xt")
            nc.sync.dma_start(xt_t[:], xt[b])
            xq_t = xq_pool.tile([4, N], BF, tag="xq")
            nc.sync.dma_start(xq_t[:], xq[b])

            # ---- m1: logits in (n, k) layout; x is the stationary.
            # Chunks interleave across two PSUM banks (A: 0,2,4,6 / B: 1,3,5)
            # so the bf16 constant rides batch in pairs and the fp8 x-chunk
            # matmuls run in uninterrupted groups of 8 (fewer dtype switches).
            NA = 4  # chunks in bank A
            lga_p = ps_lga.tile([NC_, NA * K + 1], F32, tag="lga")
            lgb_p = ps_lgb.tile([NC_, (NT - NA) * K], F32, tag="lgb")

            def chunk_out(j):
                g = j // 2
                if j % 2 == 0:
                    return lga_p[:, g * K : (g + 1) * K]
                return lgb_p[:, g * K : (g + 1) * K]

            for jp in range(0, NT, 2):
                pair = [jp] + ([jp + 1] if jp + 1 < NT else [])
                for j in pair:  # bf16 rides open each chunk's psum group
                    nc.tensor.matmul(
                        chunk_out(j),
                        xq_t[:, j * NC_ : (j + 1) * NC_],
                        sp4_t[:],
                        start=True,
                        stop=False,
                    )
                for j in pair:  # fp8 x-chunk matmuls, grouped by dtype
                    o = chunk_out(j)
                    for jc in range(CCH):
                        nc.tensor.matmul(
                            o,
                            xb_t[:, jc * N + j * NC_ : jc * N + (j + 1) * NC_],
                            w1_t[:, jc * K : (jc + 1) * K],
                            start=False,
                            stop=(jc == CCH - 1),
                        )

            # ---- softmax in (n, k): exp, denom over free dim, normalize.
            # Chunk order in E/den/at is bank-grouped: [0,2,4,6, 1,3,5].
            Ea_t = sm_pool.tile([NC_, NA * K], BF, tag="Ea")
            Eb_t = sm_pool.tile([NC_, (NT - NA) * K], BF, tag="Eb")
            nc.scalar.activation(
                Eb_t[:], lgb_p[:], mybir.ActivationFunctionType.Exp,
                scale=1.0 / W1SC,
            )
            nc.scalar.activation(
                Ea_t[:], lga_p[:, : NA * K], mybir.ActivationFunctionType.Exp,
                scale=1.0 / W1SC,
            )
            d_t = sm_pool.tile([NC_, NT], F32, tag="d")
            nc.vector.reduce_sum(
                d_t[:, :NA], Ea_t[:].rearrange("p (j k) -> p j k", k=K),
                axis=mybir.AxisListType.X,
            )
            nc.vector.reduce_sum(
                d_t[:, NA:], Eb_t[:].rearrange("p (j k) -> p j k", k=K),
                axis=mybir.AxisListType.X,
            )
            r_t = sm_pool.tile([NC_, NT], F32, tag="r")
            nc.vector.reciprocal(r_t[:], d_t[:])
            at_t = at_pool.tile([NC_, NT * K], BF, tag="at")
            nc.vector.tensor_mul(
                at_t[:, : NA * K].rearrange("p (j k) -> p j k", k=K),
                Ea_t[:].rearrange("p (j k) -> p j k", k=K),
                r_t[:, :NA].unsqueeze(-1).broadcast_to((NC_, NA, K)),
            )
            nc.vector.tensor_mul(
                at_t[:, NA * K :].rearrange("p (j k) -> p j k", k=K),
                Eb_t[:].rearrange("p (j k) -> p j k", k=K),
                r_t[:, NA:].unsqueeze(-1).broadcast_to((NC_, NT - NA, K)),
            )
            inflight.append((b, xt_t, at_t, lga_p))

        if it >= SKEW:
            b2, xt2, at2, lga2 = inflight.pop(0)
            xt2_v = xt2[:].rearrange("p (j c) -> p j c", c=C)
            wx_p = ps_wx.tile([K, C], F32, tag="wx")
            ws_p = ps_ws.tile([K, 1], F32, tag="ws")
            for j in range(NT):
                g = j // 2 if j % 2 == 0 else NA + j // 2
                lhs = at2[:, g * K : (g + 1) * K]
                nc.tensor.matmul(
                    wx_p[:],
                    lhs,
                    xt2_v[:, j],
                    start=(j == 0),
                    stop=(j == NT - 1),
                )
                nc.tensor.matmul(
                    ws_p[:],
                    lhs,
                    onec_t[:],
                    start=(j == 0),
                    stop=(j == NT - 1),
                )
            # ---- enc = (-cw)*wsum + wx.  PSUM is drained by ACT copies so
            # the DVE softmax chain never waits on PE m2; the subtract runs
            # on the otherwise-idle GpSimd from SBUF, out-DMA on Pool DGE.
            wx_sb = sb2_pool.tile([K, C], F32, tag="wxs")
            nc.scalar.copy(wx_sb[:], wx_p[:])
            ws_sb = sb2_pool.tile([K, 1], F32, tag="wss")
            nc.scalar.copy(ws_sb[:], ws_p[:])
            t_sb = sb2_pool.tile([K, C], F32, tag="ts")
            nc.scalar.mul(t_sb[:], negcw_t[:], ws_sb[:])
            o_t = out_pool.tile([K, C], BF, tag="o")
            nc.gpsimd.tensor_add(o_t[:], t_sb[:], wx_sb[:])
            nc.gpsimd.dma_start(enc[b2], o_t[:])


def host_prep(x, codewords, scale):
    """Build per-core input maps. x:(64,512,28,28) cw:(32,512) s:(32,)"""
    x = np.asarray(x, np.float32).reshape(B, C, N)
    cw = np.asarray(codewords, np.float32)
    s = np.asarray(scale, np.float32)

    s_max = float(s.max())
    sp = (s - s_max).astype(np.float32) * W1SC
    c_sq = (cw * cw).sum(-1)
    bias = (s * c_sq).astype(np.float32) * W1SC
    sph = sp.astype(BF16)
    spl = (sp - sph.astype(np.float32)).astype(BF16)
    sp4 = np.stack([sph, sph, spl, bias.astype(BF16)], axis=0)  # (4, K) bf16

    w1_full = (-2.0 * W1SC * s[None, :] * cw.T).astype(np.float32)  # (C, K)
    w1 = np.ascontiguousarray(
        w1_full.reshape(CCH, 128, K).transpose(1, 0, 2).reshape(128, CCH * K)
    ).astype(FP8)
    negcw = np.ascontiguousarray(-cw).astype(np.float32)
    onec = np.ones((NC_, 1), BF16)

    # xb[b, p, jc*N + n] = x[b, jc*128 + p, n]  (3136B contiguous per part)
    xb_all = np.ascontiguousarray(
        x.reshape(B, CCH, 128, N).transpose(0, 2, 1, 3)
    ).reshape(B, 128, CCH * N).astype(FP8)
    # xt[b, p, j*C + c] = x[b, c, j*112 + p]  (7168B contiguous per part)
    xt_all = np.ascontiguousarray(
        x.transpose(0, 2, 1).reshape(B, NT, NC_, C).transpose(0, 2, 1, 3)
    ).reshape(B, NC_, NT * C).astype(BF16)
    xsq_f32 = (x * x).sum(1).astype(np.float32)  # (B, 784)
    xh = xsq_f32.astype(BF16)
    xl = (xsq_f32 - xh.astype(np.float32)).astype(BF16)
    ones_n = np.ones_like(xh)
    xq_all = np.stack([xh, xl, xh, ones_n], axis=1)  # (B, 4, 784) bf16

    in_maps = []
    for i in range(NCORES):
        sl = slice(i * BPC, (i + 1) * BPC)
        in_maps.append(
            {
                "xb": np.ascontiguousarray(xb_all[sl]),
                "xt": np.ascontiguousarray(xt_all[sl]),
                "xq": np.ascontiguousarray(xq_all[sl]),
                "w1": w1,
                "sp4": sp4,
                "negcw": negcw,
                "onec": onec,
            }
        )
    return in_maps


_CACHED_NC = None


def _install_profile_shim():
    """Provide antenv.axon_hooks (absent in this container) so
    run_bass_kernel_spmd(trace=True) can NTFF-profile via the axon .so."""
    import sys
    import types
    import ctypes
    import contextlib

    if "antenv.axon_hooks" in sys.modules:
        return
    so_path = "/opt/axon/libaxon_pjrt.so"
    try:
        lib = ctypes.CDLL(so_path)
        if not hasattr(lib, "axon_start_nrt_profile"):
            return
    except OSError:
        return
    lib.axon_start_nrt_profile.argtypes = [
        ctypes.POINTER(ctypes.c_int64),
        ctypes.c_size_t,
    ]
    lib.axon_start_nrt_profile.restype = ctypes.c_int64
    lib.axon_stop_nrt_profile.argtypes = [ctypes.c_char_p]
    lib.axon_stop_nrt_profile.restype = ctypes.c_int64

    @contextlib.contextmanager
    def _hook(output_dir, device_ids):
        import jax

        jax.devices()
        if device_ids:
            ids = (ctypes.c_int64 * len(device_ids))(*device_ids)
            rc = lib.axon_start_nrt_profile(ids, len(device_ids))
        else:
            rc = lib.axon_start_nrt_profile(None, 0)
        if rc != 0:
            raise RuntimeError(f"axon_start_nrt_profile rc={rc}")
        try:
            yield
        finally:
            n = lib.axon_stop_nrt_profile(str(output_dir).encode())
            print(f"profile: {n} file(s) written to {output_dir}")

    mod = types.ModuleType("antenv.axon_hooks")
    mod.get_axon_ntff_profile_hook = lambda: _hook
    mod.set_axon_ntff_profile_hook = lambda h: None
    sys.modules["antenv.axon_hooks"] = mod
    import antenv

    antenv.axon_hooks = mod
    bass_utils.upload_artifacts = lambda tmpdir: "local://" + tmpdir


def kernel(x, codewords, scale):
    global _CACHED_NC, LAST_EXEC_NS, LAST_RESULTS
    if _CACHED_NC is None:
        _CACHED_NC = build_nc()
    nc = _CACHED_NC
    in_maps = host_prep(x, codewords, scale)
    trace = bool(int(os.environ.get("KERNEL_TRACE", "0")))
    if trace:
        _install_profile_shim()
    res = bass_utils.run_bass_kernel_spmd(
        nc, in_maps, list(range(NCORES)), trace=trace
    )
    LAST_EXEC_NS = res.exec_time_ns
    LAST_RESULTS = res
    out = np.concatenate(
        [np.asarray(res.results[i]["enc"]) for i in range(NCORES)], axis=0
    )
    return out.astype(np.float32)
